# revision 1
# baseline (speedup 1.0000x reference)
"""Trainium2 Bass kernel for nn_Attention_11793980194868.

Conv3d(depthwise,k3)+BN -> QKV linear -> 6-head attention -> out proj.
Sharding: data-parallel over batch B=8, one batch element per NeuronCore.

Per-core program layout (all "T" tensors channels-on-partitions):
  - x arrives host-padded+transposed: [3ci, 128c, 18, 18, 18] fp32
  - depthwise conv = 27 diagonal-weight matmuls on PE accumulating in PSUM
    (diag matrices built on DVE as identity * w[c,tap])
  - BN folded into projection weights (scale) and beta vectors (bias)
  - attention computed transposed: scores^T = K Q^T per head, exp on ACT
    (scale folded, no max-subtract: |scores*scale| < 0.2), AV with a ones
    column appended to V so PSUM row 64 carries the softmax denominators
  - softmax denominators: reciprocal on DVE, broadcast across partitions via a
    DRAM-roundtrip DMA (step-0 partition reads are only legal on DRAM APs;
    gpsimd partition_broadcast miscomputes on HW in this environment)
  - conv taps split across PE (diag matmuls) and DVE (scalar_tensor_tensor
    in-place accumulation), tunable via Q_SPLIT/KV_SPLIT
  - final proj from o^T per tq-half with proj bias added via broadcast tile
"""

import os
import numpy as np

try:
    import concourse  # noqa: F401
except ImportError:  # harness environment fallback
    import sys

    sys.path.insert(0, "/opt/trn_rl_repo")

B, T, C = 8, 4096, 384
H, DH = 6, 64
NCI = 3  # channel tiles of 128
P = 128
TKV = 512
EPS = 1e-5
SCALE = float(C) ** -0.5
PD = 18  # padded spatial extent
N_CORES = 8

_TAPS = [(kh, kw, kd) for kh in range(3) for kw in range(3) for kd in range(3)]
# conv tap split across engines: (PE, DVE, GPSIMD) counts summing to 27
Q_SPLIT = (21, 6, 0)
KV_SPLIT = (22, 5, 0)


def build_program():
    import concourse.bacc as bacc
    import concourse.tile as tile
    from concourse import library_config, mybir

    dt = mybir.dt
    f32, f32r, bf16 = dt.float32, dt.float32r, dt.bfloat16
    AF = mybir.ActivationFunctionType

    nc = bacc.Bacc(None)

    # ---- DRAM I/O (per core = one batch element) ----
    xpad_d = nc.dram_tensor("xpad", [NCI, P, PD, PD, PD], f32r, kind="ExternalInput")
    wc_d = {
        "q": nc.dram_tensor("wcq", [NCI, P, 27], f32, kind="ExternalInput"),
        "k": nc.dram_tensor("wck", [NCI, P, 27], f32, kind="ExternalInput"),
        "v": nc.dram_tensor("wcv", [NCI, P, 27], f32, kind="ExternalInput"),
    }
    ident_d = nc.dram_tensor("ident", [P, P], f32, kind="ExternalInput")
    wq_d = nc.dram_tensor("wqT", [NCI, P, C], f32r, kind="ExternalInput")
    wk_d = nc.dram_tensor("wkT", [NCI, P, C], f32r, kind="ExternalInput")
    wv_d = nc.dram_tensor("wvT", [NCI, P, C], f32r, kind="ExternalInput")
    pj_d = nc.dram_tensor("projT", [NCI, P, C], f32r, kind="ExternalInput")
    bq_d = nc.dram_tensor("betaq", [NCI, P, 1], f32, kind="ExternalInput")
    bk_d = nc.dram_tensor("betak", [NCI, P, 1], f32, kind="ExternalInput")
    bv_d = nc.dram_tensor("betav", [P, C], f32, kind="ExternalInput")
    pb_d = nc.dram_tensor("projb", [P, C], f32, kind="ExternalInput")
    invs_d = nc.dram_tensor("invscratch", [2 * H * 2, 1024], f32)
    out_d = nc.dram_tensor("out", [T, C], f32, kind="ExternalOutput")

    r = lambda ap: ap

    with nc.allow_low_precision("float32r matmul inputs"), tile.TileContext(nc) as tc:
        with (
            tc.tile_pool(name="consts", bufs=1) as cpool,
            tc.tile_pool(name="zq", bufs=NCI) as zqp,
            tc.tile_pool(name="zk", bufs=NCI) as zkp,
            tc.tile_pool(name="zv", bufs=4) as zvp,
        ):
            # ---- constants ----
            ident = cpool.tile([P, P], f32)
            nc.sync.dma_start(ident[:], ident_d[:])
            wc_sb = {}
            for cname, d in wc_d.items():
                for ci in range(NCI):
                    t = cpool.tile([P, 27], f32, tag=f"wc_{cname}_{ci}", name=f"wc_{cname}_{ci}")
                    nc.sync.dma_start(t[:], d[ci])
                    wc_sb[(cname, ci)] = t

            def load3(d, tag):
                ts = []
                for ci in range(NCI):
                    t = cpool.tile([P, C], f32r, tag=f"{tag}{ci}")
                    nc.sync.dma_start(t[:], d[ci])
                    ts.append(t)
                return ts

            wload = [None]

            # ================= phase A: conv + projections =================
            qbn, kbn, vbn = [], [], []
            with (
                tc.tile_pool(name="xp", bufs=2) as xpp,
                tc.tile_pool(name="qbn", bufs=NCI) as qbnp,
                tc.tile_pool(name="kvbn", bufs=2 * NCI) as kvbnp,
                tc.tile_pool(name="diag", bufs=32) as dgp,
                tc.tile_pool(name="psconv", bufs=2, space="PSUM") as psc,
                tc.tile_pool(name="pskv", bufs=1, space="PSUM") as pskv,
                tc.tile_pool(name="psmm", bufs=3, space="PSUM") as psmm,
            ):
                for ci in range(NCI):
                    xp = xpp.tile([P, PD, PD, PD], f32r, tag="xp")
                    for sl in range(PD):
                        nc.sync.dma_start(
                            xp[:, sl : sl + 1, :, :],
                            xpad_d[ci, :, sl : sl + 1, :, :],
                        )

                    # --- q conv (stride 1): taps split across PE/DVE/Pool ---
                    nq_pe, nq_dve, nq_pool = Q_SPLIT
                    qb = qbnp.tile([P, T], f32r, tag="qbn")
                    qbn.append(qb)
                    dgs_q = []
                    for tap in range(nq_pe):
                        dg = dgp.tile([P, P], f32r, tag="dg", name=f"dgq_{ci}_{tap}")
                        nc.vector.tensor_scalar_mul(
                            dg[:], ident[:], wc_sb[("q", ci)][:, tap : tap + 1]
                        )
                        dgs_q.append(dg)
                    for quarter in range(4):
                        qv = qb[:, quarter * 1024 : (quarter + 1) * 1024].rearrange(
                            "p (a b c) -> p a b c", a=4, b=16
                        )
                        # DVE + Pool taps accumulate in-place into qb (per h-plane:
                        # ScalarTensorTensor is limited to partition + 2 free dims)
                        for li, tap in enumerate(range(nq_pe, 27)):
                            kh, kw, kd = _TAPS[tap]
                            eng = nc.vector if li < nq_dve else nc.gpsimd
                            wsc = wc_sb[("q", ci)][:, tap : tap + 1]
                            for hh in range(4):
                                xin = xp[
                                    :,
                                    kh + quarter * 4 + hh,
                                    kw : kw + 16,
                                    kd : kd + 16,
                                ].bitcast(f32)
                                qvh = qv[:, hh, :, :]
                                if li == 0:
                                    eng.tensor_scalar_mul(qvh, xin, wsc)
                                else:
                                    eng.scalar_tensor_tensor(
                                        qvh, xin, wsc, qvh,
                                        mybir.AluOpType.mult, mybir.AluOpType.add,
                                    )
                        ps = psc.tile([P, 4, 16, 16], f32, tag="psq", name=f"psq_{ci}_{quarter}")
                        for tap in range(nq_pe):
                            kh, kw, kd = _TAPS[tap]
                            for j in range(2):  # 2-h-plane slabs = 512 = one bank
                                h0 = quarter * 4 + 2 * j
                                nc.tensor.matmul(
                                    ps[:, 2 * j : 2 * j + 2, :, :],
                                    r(dgs_q[tap][:]),
                                    r(xp[:, kh + h0 : kh + h0 + 2, kw : kw + 16, kd : kd + 16]),
                                    start=(tap == 0),
                                    stop=(tap == nq_pe - 1),
                                )
                        qv2 = qb[:, quarter * 1024 : (quarter + 1) * 1024]
                        nc.vector.tensor_add(
                            qv2, ps[:].rearrange("p a b c -> p (a b c)"), qv2
                        )

                    # --- k/v convs (stride 2) ---
                    nk_pe, nk_dve, nk_pool = KV_SPLIT
                    for name, dst_list in (("k", kbn), ("v", vbn)):
                        dst = kvbnp.tile([P, TKV], f32r, tag="kvbn", name=f"kvbn_{name}_{ci}")
                        dv = dst[:].rearrange("p (a b c) -> p a b c", a=8, b=8)
                        for li, tap in enumerate(range(nk_pe, 27)):
                            kh, kw, kd = _TAPS[tap]
                            eng = nc.vector if li < nk_dve else nc.gpsimd
                            wsc = wc_sb[(name, ci)][:, tap : tap + 1]
                            for hh in range(8):
                                xin = xp[
                                    :, kh + 2 * hh, kw : kw + 16 : 2, kd : kd + 16 : 2
                                ].bitcast(f32)
                                dvh = dv[:, hh, :, :]
                                if li == 0:
                                    eng.tensor_scalar_mul(dvh, xin, wsc)
                                else:
                                    eng.scalar_tensor_tensor(
                                        dvh, xin, wsc, dvh,
                                        mybir.AluOpType.mult, mybir.AluOpType.add,
                                    )
                        ps = pskv.tile([P, 8, 8, 8], f32, tag="pskv")
                        for tap in range(nk_pe):
                            kh, kw, kd = _TAPS[tap]
                            dg = dgp.tile([P, P], f32r, tag="dg", name=f"dg{name}_{ci}_{tap}")
                            nc.vector.tensor_scalar_mul(
                                dg[:], ident[:], wc_sb[(name, ci)][:, tap : tap + 1]
                            )
                            nc.tensor.matmul(
                                ps[:],
                                r(dg[:]),
                                r(xp[:, kh : kh + 16 : 2, kw : kw + 16 : 2, kd : kd + 16 : 2]),
                                start=(tap == 0),
                                stop=(tap == nk_pe - 1),
                            )
                        dv2 = dst[:]
                        nc.vector.tensor_add(
                            dv2, ps[:].rearrange("p a b c -> p (a b c)"), dv2
                        )
                        dst_list.append(dst)

                # ---- deferred weight loads (after conv DMAs got queue priority) ----
                wq_sb = load3(wq_d, "wq")
                wk_sb = load3(wk_d, "wk")
                wv_sb = load3(wv_d, "wv")
                pj_sb = load3(pj_d, "pj")
                bq_sb, bk_sb = [], []
                for ci in range(NCI):
                    t = cpool.tile([P, 1], f32, tag=f"bq{ci}", name=f"bq_{ci}")
                    nc.sync.dma_start(t[:], bq_d[ci])
                    bq_sb.append(t)
                    t = cpool.tile([P, 1], f32, tag=f"bk{ci}", name=f"bk_{ci}")
                    nc.sync.dma_start(t[:], bk_d[ci])
                    bk_sb.append(t)
                bv_bc = cpool.tile([P, C], f32, tag="bvbc")
                nc.sync.dma_start(bv_bc[:], bv_d[:])
                pb_bc = cpool.tile([P, C], f32, tag="pbbc")
                nc.sync.dma_start(pb_bc[:], pb_d[:])

                # ---- q projection: z_q^T[c_out, t] ----
                zq = []
                for m in range(NCI):
                    z = zqp.tile([P, T], f32r, tag="zq")
                    zq.append(z)
                    for ch in range(8):
                        ps = psmm.tile([P, TKV], f32, tag="mm")
                        for kci in range(NCI):
                            nc.tensor.matmul(
                                ps[:],
                                r(wq_sb[kci][:, m * P : (m + 1) * P]),
                                r(qbn[kci][:, ch * TKV : (ch + 1) * TKV]),
                                start=(kci == 0),
                                stop=(kci == NCI - 1),
                            )
                        nc.scalar.activation(
                            z[:, ch * TKV : (ch + 1) * TKV],
                            ps[:],
                            AF.Identity,
                            bias=bq_sb[m][:, 0:1],
                        )

                # ---- k projection: z_k^T[c_out, tkv] ----
                zk = []
                for m in range(NCI):
                    z = zkp.tile([P, TKV], f32r, tag="zk")
                    zk.append(z)
                    ps = psmm.tile([P, TKV], f32, tag="mm")
                    for kci in range(NCI):
                        nc.tensor.matmul(
                            ps[:],
                            r(wk_sb[kci][:, m * P : (m + 1) * P]),
                            r(kbn[kci][:]),
                            start=(kci == 0),
                            stop=(kci == NCI - 1),
                        )
                    nc.scalar.activation(
                        z[:], ps[:], AF.Identity, bias=bk_sb[m][:, 0:1]
                    )

                # ---- v projection: z_v[tkv, (h, dh)] bf16 + ones column ----
                zv = []
                for mt in range(4):
                    zt = zvp.tile([P, H, DH + 1], bf16, tag="zv")
                    zv.append(zt)
                    ps = psmm.tile([P, TKV], f32, tag="mm")
                    for kci in range(NCI):
                        nc.tensor.matmul(
                            ps[:, 0:C],
                            r(vbn[kci][:, mt * P : (mt + 1) * P]),
                            r(wv_sb[kci][:]),
                            start=(kci == 0),
                            stop=(kci == NCI - 1),
                        )
                    nc.vector.tensor_add(
                        zt[:, :, 0:DH],
                        ps[:, 0:C].rearrange("p (h d) -> p h d", h=H),
                        bv_bc[:].rearrange("p (h d) -> p h d", h=H),
                    )
                    nc.vector.memset(zt[:, :, DH : DH + 1], 1.0)

            # ================= phase B: attention + out proj =================
            oT = []
            with (
                tc.tile_pool(name="at", bufs=2) as atp,
                tc.tile_pool(name="oT", bufs=NCI) as otp,
                tc.tile_pool(name="inv", bufs=2) as invp,
                tc.tile_pool(name="ostage", bufs=3) as osp,
                tc.tile_pool(name="outt", bufs=8) as outp,
                tc.tile_pool(name="psav", bufs=2, space="PSUM") as psav,
                tc.tile_pool(name="psmmB", bufs=2, space="PSUM") as psmmb,
            ):
                for hp in range(NCI):
                    o = otp.tile([P, T], f32r, tag="oT")
                    oT.append(o)
                HTQ = 2048  # tq half size
                for half in range(2):
                    for h in (1, 0, 3, 2, 5, 4):  # odd heads (DMA-shifted) first
                        hp, sub = h // 2, (h % 2) * 64
                        at = atp.tile([P, 4, HTQ], bf16, tag="at")
                        for tkt in range(4):
                            for tqc in range(4):
                                ps = psmmb.tile([P, TKV], f32, tag="qk")
                                off = half * HTQ + tqc * TKV
                                nc.tensor.matmul(
                                    ps[:],
                                    r(zk[hp][sub : sub + 64, tkt * P : (tkt + 1) * P]),
                                    r(zq[hp][sub : sub + 64, off : off + TKV]),
                                    start=True,
                                    stop=True,
                                )
                                nc.scalar.activation(
                                    at[:, tkt, tqc * TKV : (tqc + 1) * TKV],
                                    ps[:],
                                    AF.Exp,
                                    scale=SCALE,
                                )
                        # AV with ones column: rows 0:64 = o_un, row 64 = denom
                        for q2 in range(2):  # 1024-wide chunks (2 PSUM banks)
                            po = psav.tile([P, 1024], f32, tag="av")
                            for tkt in range(4):
                                for tqc in range(2):
                                    o0 = q2 * 1024 + tqc * TKV
                                    nc.tensor.matmul(
                                        po[0:65, tqc * TKV : (tqc + 1) * TKV],
                                        zv[tkt][:, h, :],
                                        at[:, tkt, o0 : o0 + TKV],
                                        start=(tkt == 0),
                                        stop=(tkt == 3),
                                    )
                            stg = osp.tile([P, 1024], f32r, tag="ost")
                            nc.vector.reciprocal(stg[64:65, :], po[64:65, :])
                            idx = (half * H + h) * 2 + q2
                            nc.sync.dma_start(
                                invs_d[idx : idx + 1, :], stg[64:65, :].bitcast(f32)
                            )
                            invb = invp.tile([P, 1024], f32, tag="invb")
                            nc.sync.dma_start(
                                invb[0:64, :], invs_d[idx, :].partition_broadcast(64)
                            )
                            off = half * HTQ + q2 * 1024
                            if sub == 0:
                                nc.vector.tensor_mul(
                                    oT[hp][0:64, off : off + 1024],
                                    po[0:64, :],
                                    invb[0:64, :],
                                )
                            else:
                                nc.vector.tensor_mul(
                                    stg[0:64, :], po[0:64, :], invb[0:64, :]
                                )
                                nc.sync.dma_start(
                                    oT[hp][64:128, off : off + 1024], stg[0:64, :]
                                )

                    # ---- output projection for this tq half ----
                    for tt in range(half * 16, (half + 1) * 16):
                        ps = psmmb.tile([P, TKV], f32, tag="pj", name=f"psproj_{tt}")
                        for kci in range(NCI):
                            nc.tensor.matmul(
                                ps[:, 0:C],
                                r(oT[kci][:, tt * P : (tt + 1) * P]),
                                r(pj_sb[kci][:]),
                                start=(kci == 0),
                                stop=(kci == NCI - 1),
                            )
                        ot = outp.tile([P, C], f32, tag="outt", name=f"ot_{tt}")
                        nc.vector.tensor_add(ot[:], ps[:, 0:C], pb_bc[:])
                        nc.sync.dma_start(out_d[tt * P : (tt + 1) * P, :], ot[:])


    nc.compile()
    return nc


def host_prep(inputs):
    """Fold BN, transpose/pad x, build per-core input maps."""
    f32 = np.float32
    x = np.asarray(inputs["x"], dtype=f32)

    def fold(p):
        g = np.asarray(inputs[f"bn_{p}_g"], f32)
        b = np.asarray(inputs[f"bn_{p}_b"], f32)
        m = np.asarray(inputs[f"bn_{p}_m"], f32)
        v = np.asarray(inputs[f"bn_{p}_v"], f32)
        a = g / np.sqrt(v + EPS)
        return a, b - m * a

    aq, bq = fold("q")
    ak, bk = fold("k")
    av_, bv = fold("v")

    wq = np.asarray(inputs["wq"], f32)
    wk = np.asarray(inputs["wk"], f32)
    wv = np.asarray(inputs["wv"], f32)
    pw = np.asarray(inputs["proj_w"], f32)
    pb = np.asarray(inputs["proj_b"], f32)

    common = {
        "wcq": np.ascontiguousarray(np.asarray(inputs["conv_q_w"], f32).reshape(NCI, P, 27)),
        "wck": np.ascontiguousarray(np.asarray(inputs["conv_k_w"], f32).reshape(NCI, P, 27)),
        "wcv": np.ascontiguousarray(np.asarray(inputs["conv_v_w"], f32).reshape(NCI, P, 27)),
        "ident": np.eye(P, dtype=f32),
        "wqT": np.ascontiguousarray((wq * aq[None, :]).T.reshape(NCI, P, C)),
        "wkT": np.ascontiguousarray((wk * ak[None, :]).T.reshape(NCI, P, C)),
        "wvT": np.ascontiguousarray((wv * av_[None, :]).T.reshape(NCI, P, C)),
        "projT": np.ascontiguousarray(pw.T.reshape(NCI, P, C)),
        "betaq": (wq @ bq).astype(f32).reshape(NCI, P, 1),
        "betak": (wk @ bk).astype(f32).reshape(NCI, P, 1),
        "betav": np.broadcast_to((wv @ bv).astype(f32), (P, C)).copy(),
        "projb": np.broadcast_to(pb.astype(f32), (P, C)).copy(),
    }

    # x: [B, T, C] -> per-batch channels-on-partitions, host-padded
    xt = x.transpose(0, 2, 1).reshape(B, NCI, P, 16, 16, 16)
    xpad = np.zeros((B, NCI, P, PD, PD, PD), f32)
    xpad[:, :, :, 1:17, 1:17, 1:17] = xt

    in_maps = []
    for b in range(B):
        m = dict(common)
        m["xpad"] = np.ascontiguousarray(xpad[b])
        in_maps.append(m)
    return in_maps


_CACHE = {}


def kernel(**inputs) -> np.ndarray:
    from concourse.bass_utils import run_bass_kernel_spmd

    if "nc" not in _CACHE:
        _CACHE["nc"] = build_program()
    nc = _CACHE["nc"]

    in_maps = host_prep(inputs)
    res = run_bass_kernel_spmd(
        nc,
        in_maps,
        core_ids=list(range(N_CORES)),
        trace=bool(int(os.environ.get("KERNEL_TRACE", "0"))),
    )
    out = np.stack([res.results[b]["out"] for b in range(B)], axis=0)
    _CACHE["last_result"] = res
    return out

